# revision 57
# baseline (speedup 1.0000x reference)
"""Trainium2 Bass kernel for a Conformer block (B=8, S=1024, D=512).

Sharding: data-parallel over batch — 1 batch element per NeuronCore, 8 cores,
no collectives.

v2 layout strategy (vs the f32r baseline):
- Residual stream x stays f32r feature-major [D, S]; all heavy matmuls run
  in bf16 with 1024-wide moving operands (xs = x * rstd_bcast cast to bf16;
  mean subtraction rides each dense matmul as one bf16 rank-1 "aug" row).
- Softmax/LN reciprocals use the single-op DVE reciprocal_approx_fast
  (the HW iterative-divide RECIPROCAL was 4us per row).
- LN variance squares run on the idle GPSIMD engine.
- Depthwise conv taps are split across PE (diagonal-matrix matmuls with
  host-built diag weights), DVE (aligned scalar_tensor_tensor in bf16 2x
  mode via an odd-shifted copy of the padded GLU output), and GPSIMD.
- Attention output tiles are packed two heads per 128-partition tile so the
  o-projection contracts 128-deep; softmax denominators come from a ones
  column in V and are normalized per whole head row.
"""

import os
import numpy as np

# ---------------- problem constants (hardcoded) ----------------
B, S, D = 8, 1024, 512
H, DH = 8, 64
FFI, CI, KCONV = 1024, 1024, 31
EPS = 1e-5
NCORES = 8
PAD = (KCONV - 1) // 2  # 15
NDC = D // 128    # 4  d-chunks
NTC = S // 128    # 8  t-chunks
HALF = S // 2     # 512
NCC = CI // 128   # 8  conv channel chunks

N_PE_TAPS = int(os.environ.get("CONF_PE_TAPS", "21"))
N_GPS_TAPS = int(os.environ.get("CONF_GPS_TAPS", "0"))
DBG = os.environ.get("CONF_DEBUG_STAGES", "0") == "1"

_ODD = list(range(1, KCONV, 2))    # 15 taps
_EVEN = list(range(0, KCONV, 2))   # 16 taps
TAPS_PE = (_ODD + _EVEN)[:N_PE_TAPS]
_rest = [j for j in range(KCONV) if j not in TAPS_PE]
TAPS_GPS = _rest[:N_GPS_TAPS]
_used = set(TAPS_PE) | set(TAPS_GPS)
TAPS_DVE = [j for j in range(KCONV) if j not in _used]
NEED_ODD = any(j % 2 == 1 for j in TAPS_DVE)


# ---------------- tile-framework workaround ----------------
def _patch_tile_drain():
    """This walrus build rejects >1 sync-wait on TPB_CTRL (Drain/NOP)
    instructions; spread the TileContext tail-drain waits across
    single-wait NOPs."""
    import concourse.tile as tile
    from concourse.vector_clock import ScopedClock
    from concourse import mybir

    if getattr(tile.TileContext, "_drain_patched", False):
        return

    def _drain_and_barrier(self, tick_clock, wait_clock):
        nc = self.nc
        carrier = nc.sync.nop(nofuse=True, hint="tail_wait_carrier")
        wait_clock.add_sem_waits(
            carrier.ins, ScopedClock({None: tick_clock.global_clock})
        )
        waits = list(carrier.ins.sync_info.on_wait)
        if len(waits) > 1:
            carrier.ins.sync_info.on_wait = waits[:1]
            for w in waits[1:]:
                nxt = nc.sync.nop(nofuse=True, hint="tail_wait_carrier")
                nxt.ins.sync_info = mybir.SyncInfo(on_wait=[w], on_update=[])
        nc.sync.drain()
        nc.all_engine_barrier()
        assert self.sems is not None
        popped = nc._tile_sem_poison_stack.pop()
        assert popped is self._sem_poison
        nc.clear_and_free_semaphores(list(self.sems.allocated().values()))
        nc.all_engine_barrier()

    tile.TileContext._drain_and_barrier = _drain_and_barrier
    tile.TileContext._drain_patched = True


# ---------------- host-side weight preparation ----------------
def _pmajor_lhsT(w, nk, nm):
    """[K, M] -> [128(p), nm, nk, 128(m)] contiguous bf16 (partition-major)."""
    import ml_dtypes
    K, M = w.shape
    assert K == nk * 128 and M == nm * 128
    return np.ascontiguousarray(
        w.reshape(nk, 128, nm, 128).transpose(1, 2, 0, 3)
    ).astype(ml_dtypes.bfloat16)


def _blob_bias(b, nm):
    """[M] -> [128, nm] f32 (per-partition bias columns)."""
    return np.ascontiguousarray(b.reshape(nm, 128).T).astype(np.float32)


def prep_inputs(inp):
    """Fold LN gains/biases, attention scale, BatchNorm, and FF 0.5 scales
    into bf16 weights. Returns dict of DRAM arrays shared by all cores."""
    import ml_dtypes
    BF = ml_dtypes.bfloat16
    f64 = lambda a: np.asarray(a, np.float64)
    out = {}

    def ln_matmul_group(pfx, g, lb, w, wb, nm, scale=1.0):
        wg = f64(w) * f64(g)[:, None] * scale
        out[f"{pfx}_w"] = _pmajor_lhsT(wg, NDC, nm)
        bias = (f64(wb) + f64(lb) @ f64(w)) * scale
        out[f"{pfx}_b"] = _blob_bias(bias, nm)

    # FF1 / FF2
    for pfx, p in [("ff1", "ff1"), ("ff2", "ff2")]:
        ln_matmul_group(f"{pfx}a", inp[f"{p}_ln_g"], inp[f"{p}_ln_b"],
                        inp[f"{p}_w1"], inp[f"{p}_b1"], FFI // 128)
        out[f"{pfx}b_w"] = _pmajor_lhsT(f64(inp[f"{p}_w2"]) * 0.5,
                                        FFI // 128, NDC)
        out[f"{pfx}b_b"] = _blob_bias(f64(inp[f"{p}_b2"]) * 0.5, NDC)

    # attention
    ln_matmul_group("wq", inp["attn_ln_g"], inp["attn_ln_b"],
                    inp["q_w"], inp["q_b"], NDC, scale=DH ** -0.5)
    ln_matmul_group("wk", inp["attn_ln_g"], inp["attn_ln_b"],
                    inp["k_w"], inp["k_b"], NDC)
    # v: rhs layout [128(p), kc, H*DH]
    wvg = f64(inp["v_w"]) * f64(inp["attn_ln_g"])[:, None]
    out["wv_w"] = np.ascontiguousarray(
        wvg.reshape(NDC, 128, H * DH).transpose(1, 0, 2)
    ).astype(BF)
    vb = f64(inp["v_b"]) + f64(inp["attn_ln_b"]) @ f64(inp["v_w"])
    out["wv_bias"] = vb.astype(BF).reshape(1, H * DH)
    out["_has_vb"] = bool(np.abs(vb).max() > 0)
    # o: [128(p = head-pair dh), 4(i), 4(mc), 128(m)]
    out["wo_w"] = np.ascontiguousarray(
        f64(inp["o_w"]).reshape(4, 128, NDC, 128).transpose(1, 0, 2, 3)
    ).astype(BF)
    out["wo_b"] = _blob_bias(f64(inp["o_b"]), NDC)

    # conv module
    ln_matmul_group("pw1", inp["conv_ln_g"], inp["conv_ln_b"],
                    inp["pw1_w"], inp["pw1_b"], 2 * CI // 128)
    inv = f64(inp["bn_g"]) / np.sqrt(f64(inp["bn_var"]) + EPS)
    dwf = f64(inp["dw_w"])[:, 0, :] * inv[:, None]  # [CI, K]
    out["dw_w"] = np.ascontiguousarray(
        dwf.reshape(NCC, 128, KCONV).transpose(1, 0, 2)
    ).astype(np.float32)   # [128, pc, K]
    cb = (f64(inp["dw_b"]) - f64(inp["bn_mean"])) * inv + f64(inp["bn_b"])
    out["dw_b"] = _blob_bias(cb, NCC)
    if TAPS_PE:
        # diag blobs [128(p), pc, jj, 128(m)]: diag(dwf[pc*128+p, tap])
        dg = np.zeros((128, NCC, len(TAPS_PE), 128), np.float32)
        for pc in range(NCC):
            for jj, j in enumerate(TAPS_PE):
                w_col = dwf[pc * 128:(pc + 1) * 128, j]
                dg[np.arange(128), pc, jj, np.arange(128)] = w_col
        out["dw_diag"] = dg.astype(BF)
    out["pw2_w"] = _pmajor_lhsT(f64(inp["pw2_w"]), NCC, NDC)
    out["pw2_b"] = _blob_bias(f64(inp["pw2_b"]), NDC)

    # final LN
    out["outln_g"] = _blob_bias(f64(inp["out_ln_g"]), NDC)
    out["outln_b"] = _blob_bias(f64(inp["out_ln_b"]), NDC)
    out["_triv_final"] = bool(
        np.allclose(inp["out_ln_g"], 1.0) and np.allclose(inp["out_ln_b"], 0.0)
    )
    out["ident"] = np.eye(128, dtype=np.float32)
    out["ident_bf"] = np.eye(128, dtype=np.float32).astype(BF)
    out["ones_f"] = np.ones((128, 1), dtype=np.float32)
    out["ones_bf"] = np.ones((1, 128), dtype=np.float32).astype(BF)
    out["ones_bc"] = np.ones((128, 1), dtype=np.float32).astype(BF)
    return out


# ---------------- kernel builder ----------------
def build_program():
    _patch_tile_drain()
    import concourse.bass as bass
    import concourse.tile as tile
    from concourse import mybir
    from contextlib import ExitStack

    dt = mybir.dt
    AF = mybir.ActivationFunctionType
    OP = mybir.AluOpType
    F32 = dt.float32
    F32R = dt.float32r
    BF16 = dt.bfloat16

    nc = bass.Bass("TRN2", target_bir_lowering=False, debug=False)

    # ---- DRAM declarations ----
    x_d = nc.dram_tensor("x", [S, D], F32R, kind="ExternalInput")
    y_d = nc.dram_tensor("y", [S, D], F32, kind="ExternalOutput")
    dram = {}

    def din(name, shape, dtp):
        dram[name] = nc.dram_tensor(name, list(shape), dtp,
                                    kind="ExternalInput")
        return dram[name]

    din("ident", [128, 128], F32R)
    din("ident_bf", [128, 128], BF16)
    din("ones_f", [128, 1], F32R)
    din("ones_bf", [1, 128], BF16)
    din("ones_bc", [128, 1], BF16)
    for pfx, nm in [("ff1a", FFI // 128), ("wq", NDC), ("wk", NDC),
                    ("pw1", 2 * CI // 128), ("ff2a", FFI // 128)]:
        din(f"{pfx}_w", [128, nm, NDC, 128], BF16)
        din(f"{pfx}_b", [128, nm], F32)
    for pfx, nk, nm in [("ff1b", FFI // 128, NDC), ("pw2", NCC, NDC),
                        ("ff2b", FFI // 128, NDC)]:
        din(f"{pfx}_w", [128, nm, nk, 128], BF16)
        din(f"{pfx}_b", [128, nm], F32)
    din("wv_w", [128, NDC, H * DH], BF16)
    din("wv_bias", [1, H * DH], BF16)
    din("wo_w", [128, 4, NDC, 128], BF16)
    din("wo_b", [128, NDC], F32)
    din("dw_w", [128, NCC, KCONV], F32)
    din("dw_b", [128, NCC], F32)
    if TAPS_PE:
        din("dw_diag", [128, NCC, len(TAPS_PE), 128], BF16)
    din("outln_g", [128, NDC], F32)
    din("outln_b", [128, NDC], F32)

    dbg_d = {}
    if DBG:
        for s_ in ["ff1", "attn", "conv", "ff2"]:
            dbg_d[s_] = nc.dram_tensor(f"dbg_{s_}", [D, S], F32R,
                                       kind="ExternalOutput")

    HAS_VB = build_program._has_vb
    TRIV_FINAL = build_program._triv_final

    with tile.TileContext(nc) as tc, ExitStack() as top:
        top.enter_context(nc.allow_low_precision(
            reason="bf16 compute is intentional; gate is 2e-2"))
        # ---- persistent pools ----
        p_x = top.enter_context(tc.tile_pool(name="p_x", bufs=1))
        p_const = top.enter_context(tc.tile_pool(name="p_const", bufs=1))
        p_rows = top.enter_context(tc.tile_pool(name="p_rows", bufs=1))
        p_w = top.enter_context(tc.tile_pool(name="p_w", bufs=1))
        p_ffw = top.enter_context(tc.tile_pool(name="p_ffw", bufs=1))
        p_pw1 = top.enter_context(tc.tile_pool(name="p_pw1", bufs=3))
        p_bias = top.enter_context(tc.tile_pool(name="p_bias", bufs=1))
        p_xs = top.enter_context(tc.tile_pool(name="p_xs", bufs=4))
        ps_big = top.enter_context(
            tc.tile_pool(name="ps_big", bufs=3, space="PSUM"))
        ps_st = top.enter_context(
            tc.tile_pool(name="ps_st", bufs=2, space="PSUM"))

        ident = p_const.tile([128, 128], F32R, tag="ident", name="ident")
        nc.sync.dma_start(out=ident, in_=dram["ident"].ap())
        ident_bf = p_const.tile([128, 128], BF16, tag="identb", name="identb")
        nc.sync.dma_start(out=ident_bf, in_=dram["ident_bf"].ap())
        ones_f = p_const.tile([128, 1], F32R, tag="onesf", name="onesf")
        nc.sync.dma_start(out=ones_f, in_=dram["ones_f"].ap())
        ones_bf = p_const.tile([1, 128], BF16, tag="onesb", name="onesb")
        nc.sync.dma_start(out=ones_bf, in_=dram["ones_bf"].ap())
        ones_bc = p_const.tile([128, 1], BF16, tag="onesbc", name="onesbc")
        nc.sync.dma_start(out=ones_bc, in_=dram["ones_bc"].ap())
        epst = p_const.tile([128, 1], F32, tag="epst", name="epst")
        nc.vector.memset(epst, EPS)

        # ---- resident weights (one DMA per blob) ----
        W = {}

        def wload(name, shape, dtp=BF16):
            t = p_w.tile(list(shape), dtp, tag=name, name=name)
            nc.sync.dma_start(out=t, in_=dram[name].ap())
            W[name] = t
            return t

        for pfx, nm in [("wq", NDC), ("wk", NDC)]:
            wload(f"{pfx}_w", [128, nm, NDC, 128])
        wload("pw2_w", [128, NDC, NCC, 128])
        wload("wv_w", [128, NDC, H * DH])
        wload("wo_w", [128, 4, NDC, 128])
        wload("dw_w", [128, NCC, KCONV], F32)

        if HAS_VB:
            wvb = p_bias.tile([1, H * DH], BF16, tag="wv_bias", name="wv_bias")
            nc.sync.dma_start(out=wvb, in_=dram["wv_bias"].ap())

        BIA = {}
        for name, nm in [("ff1a_b", FFI // 128), ("ff1b_b", NDC),
                         ("wq_b", NDC), ("wk_b", NDC), ("wo_b", NDC),
                         ("pw1_b", 2 * CI // 128), ("dw_b", NCC),
                         ("pw2_b", NDC), ("ff2a_b", FFI // 128),
                         ("ff2b_b", NDC), ("outln_g", NDC), ("outln_b", NDC)]:
            t = p_bias.tile([128, nm], F32, tag=name, name=name)
            nc.sync.dma_start(out=t, in_=dram[name].ap())
            BIA[name] = t

        x_t = [p_x.tile([128, S], F32R, tag=f"x{i}", name=f"x{i}")
               for i in range(NDC)]

        # ---- load + transpose x into feature-major ----
        with tc.tile_pool(name="p_xin", bufs=8) as p_xin:
            xin = []
            for tck in range(NTC):
                t = p_xin.tile([128, D], F32R, tag="xin", name="xin")
                nc.sync.dma_start(out=t, in_=x_d[tck * 128:(tck + 1) * 128, :])
                xin.append(t)
            for mc in range(NDC):
                for th in range(2):
                    pt = ps_st.tile([128, HALF], F32R, tag="st", name="pt")
                    for q in range(4):
                        nc.tensor.transpose(
                            out=pt[:, q * 128:(q + 1) * 128],
                            in_=xin[th * 4 + q][:, mc * 128:(mc + 1) * 128],
                            identity=ident,
                        )
                    nc.scalar.copy(
                        out=x_t[mc][:, th * HALF:(th + 1) * HALF], in_=pt)

        # ---- LN stats: xs = (x - mean) * rstd, bf16 ----
        def ln_stats():
            rstd = p_rows.tile([1, S], F32, tag="rstd", name="rstd")
            mean = p_rows.tile([1, S], F32, tag="mean", name="mean")
            c1 = 1.0 / D
            # bf16 copy of x: stats matmuls run at bf16 rate, f32r runs 2x
            xb = [p_xs.tile([128, S], BF16, tag="xb", name="xb")
                  for _ in range(NDC)]
            for kc in range(NDC):
                nc.vector.tensor_copy(out=xb[kc], in_=x_t[kc])
            with tc.tile_pool(name="p_sq", bufs=3) as p_sq:
                for th in range(2):
                    sl = slice(th * HALF, (th + 1) * HALF)
                    s1 = ps_st.tile([1, HALF], F32, tag="st", name="s1")
                    s2 = ps_st.tile([1, HALF], F32, tag="st", name="s2")
                    for kc in range(NDC):
                        nc.tensor.matmul(s1, ones_bc, xb[kc][:, sl],
                                         start=(kc == 0), stop=(kc == NDC - 1))
                    for kc in range(NDC):
                        sq = p_sq.tile([128, HALF], BF16, tag="sq", name="sq")
                        nc.gpsimd.tensor_tensor(
                            out=sq, in0=xb[kc][:, sl], in1=xb[kc][:, sl],
                            op=OP.mult)
                        nc.tensor.matmul(s2, ones_bc, sq,
                                         start=(kc == 0), stop=(kc == NDC - 1))
                    mean_s = mean[:, sl]
                    nc.vector.tensor_scalar_mul(out=mean_s, in0=s1, scalar1=c1)
                    msq = p_rows.tile([1, HALF], F32, tag="tmp", name="msq",
                                      bufs=4)
                    nc.vector.tensor_tensor(out=msq, in0=mean_s, in1=mean_s,
                                            op=OP.mult)
                    vpe = p_rows.tile([1, HALF], F32, tag="tmp", name="vpe",
                                      bufs=4)
                    nc.vector.scalar_tensor_tensor(
                        out=vpe, in0=s2, scalar=c1, in1=msq,
                        op0=OP.mult, op1=OP.subtract)
                    # rstd = rsqrt(v+eps) = exp(-0.5*ln(v+eps)); ln and exp
                    # share one ACT table set (also shared with softmax exp)
                    nc.scalar.activation(out=vpe, in_=vpe, func=AF.Ln,
                                         bias=epst[0:1, :], scale=1.0)
                    nc.scalar.activation(out=rstd[:, sl], in_=vpe,
                                         func=AF.Exp, scale=-0.5)
            rstdb = p_rows.tile([1, S], BF16, tag="rstdb", name="rstdb")
            nc.scalar.copy(out=rstdb, in_=rstd)
            meanb = p_rows.tile([1, S], BF16, tag="meanb", name="meanb")
            nc.scalar.copy(out=meanb, in_=mean)
            # pb = bcast(rstd), pm = bcast(mean); xs = (x - pm) * pb
            pb = ps_big.tile([128, S], F32, tag="ps", name="pb")
            pm = ps_big.tile([128, S], F32, tag="ps", name="pm")
            for th in range(2):
                sl = slice(th * HALF, (th + 1) * HALF)
                nc.tensor.matmul(pb[:, sl], ones_bf, rstdb[:, sl],
                                 start=True, stop=True)
                nc.tensor.matmul(pm[:, sl], ones_bf, meanb[:, sl],
                                 start=True, stop=True)
            xs = [p_xs.tile([128, S], BF16, tag="xs", name="xs")
                  for _ in range(NDC)]
            for kc in range(NDC):
                nc.vector.tensor_tensor(out=xs[kc], in0=xb[kc], in1=pm,
                                        op=OP.subtract)
                nc.vector.tensor_tensor(out=xs[kc], in0=xs[kc], in1=pb,
                                        op=OP.mult)
            return xs

        # ---- matmul with wide rhs: emit per-512 halves (PSUM bank limit) ----
        def mmh(ps, lhsT, rhs, start, stop, tp=None):
            n = rhs.shape[-1]
            if n <= HALF:
                nc.tensor.matmul(ps, lhsT, rhs, start=start, stop=stop,
                                 tile_position=tp)
                return
            for th in range(0, n, HALF):
                nc.tensor.matmul(ps[:, th:th + HALF], lhsT,
                                 rhs[:, th:th + HALF], start=start, stop=stop,
                                 tile_position=tp)

        def dense(wt, nk, nm, rhs_tiles, evict):
            for mc in range(nm):
                ps = ps_big.tile([128, S], F32, tag="ps", name="ps")
                for kc in range(nk):
                    mmh(ps, wt[:, mc, kc, :], rhs_tiles[kc],
                        start=(kc == 0), stop=(kc == nk - 1))
                evict(ps, mc)

        def resid_evict(bias):
            def ev(ps, mc):
                nc.vector.scalar_tensor_tensor(
                    out=x_t[mc], in0=ps, scalar=bias[:, mc:mc + 1],
                    in1=x_t[mc], op0=OP.add, op1=OP.add)
            return ev

        def dbg_dump(name):
            if DBG:
                for mc in range(NDC):
                    nc.sync.dma_start(
                        out=dbg_d[name][mc * 128:(mc + 1) * 128, :],
                        in_=x_t[mc])

        # ================= FF module =================
        def ff_module(pa, pb_):
            # stream this FF's weights into the shared slots (ff1/ff2 reuse)
            wa = p_ffw.tile([128, FFI // 128, NDC, 128], BF16, tag="wa",
                            name="wa")
            nc.sync.dma_start(out=wa, in_=dram[f"{pa}_w"].ap())
            wb = p_ffw.tile([128, NDC, FFI // 128, 128], BF16, tag="wb",
                            name="wb")
            nc.sync.dma_start(out=wb, in_=dram[f"{pb_}_w"].ap())
            xs = ln_stats()
            bt1 = BIA[f"{pa}_b"]
            with tc.tile_pool(name="p_h", bufs=FFI // 128) as p_h:
                h = [None] * (FFI // 128)

                def ev1(ps, mc):
                    h[mc] = p_h.tile([128, S], BF16, tag="h", name="h")
                    nc.scalar.activation(out=h[mc], in_=ps, func=AF.Silu,
                                         bias=bt1[:, mc:mc + 1], scale=1.0)

                dense(wa, NDC, FFI // 128, xs, ev1)
                dense(wb, FFI // 128, NDC, h, resid_evict(BIA[f"{pb_}_b"]))

        # ================= attention =================
        def attn_module():
            xs = ln_stats()
            with ExitStack() as ph:
                p_qk = ph.enter_context(tc.tile_pool(name="p_qk", bufs=16))
                p_v = ph.enter_context(tc.tile_pool(name="p_v", bufs=NTC))
                p_e = ph.enter_context(tc.tile_pool(name="p_e", bufs=8))
                p_ao = ph.enter_context(tc.tile_pool(name="p_ao", bufs=4))
                p_sm = ph.enter_context(tc.tile_pool(name="p_sm", bufs=4))

                # per-head zero-padded q/k tiles: head h's 64 dh dims live in
                # rows 0-63, rows 64-127 are zero, so the scores matmul
                # contracts a full 128 partitions (64-row tiles run 2x slower)
                q_t = [p_qk.tile([128, S], BF16, tag="qk", name="q")
                       for _ in range(H)]
                k_t = [p_qk.tile([128, S], BF16, tag="qk", name="k")
                       for _ in range(H)]
                for h_ in range(H):
                    nc.vector.memset(q_t[h_][64:128, :], 0.0)
                    nc.vector.memset(k_t[h_][64:128, :], 0.0)
                btq, btk = BIA["wq_b"], BIA["wk_b"]

                def evq(ps, mc):
                    for sub in range(2):
                        nc.vector.tensor_scalar_add(
                            out=q_t[2 * mc + sub][0:64, :],
                            in0=ps[64 * sub:64 * sub + 64, :],
                            scalar1=btq[64 * sub:64 * sub + 64, mc:mc + 1])

                def evk(ps, mc):
                    for sub in range(2):
                        nc.vector.tensor_scalar_add(
                            out=k_t[2 * mc + sub][0:64, :],
                            in0=ps[64 * sub:64 * sub + 64, :],
                            scalar1=btk[64 * sub:64 * sub + 64, mc:mc + 1])

                dense(W["wq_w"], NDC, NDC, xs, evq)
                dense(W["wk_w"], NDC, NDC, xs, evk)

                # v (token-major, ones column per head for softmax denom)
                wv = W["wv_w"]
                v_t = []
                for tck in range(NTC):
                    vt = p_v.tile([128, H, DH + 1], BF16, tag="v", name="v")
                    nc.vector.memset(vt[:, :, DH:DH + 1], 1.0)
                    pv = ps_st.tile([128, H * DH], F32, tag="st", name="pv")
                    tsl = slice(tck * 128, (tck + 1) * 128)
                    for kc in range(NDC):
                        nc.tensor.matmul(pv, xs[kc][:, tsl], wv[:, kc, :],
                                         start=(kc == 0),
                                         stop=(kc == NDC - 1 and not HAS_VB))
                    if HAS_VB:
                        nc.tensor.matmul(pv, ones_bf, wvb,
                                         start=False, stop=True)
                    nc.scalar.copy(
                        out=vt[:, :, 0:DH],
                        in_=pv.rearrange("p (h d) -> p h d", h=H))
                    v_t.append(vt)

                # per head: scores -> exp -> AV -> normalize
                ao_st = [p_ao.tile([128, S], BF16, tag="ao", name="ao")
                         for _ in range(4)]
                for h_ in range(H):
                    e_t = []
                    for ktc in range(NTC):
                        et = p_e.tile([128, S], BF16, tag="e", name="e")
                        ksl = slice(ktc * 128, (ktc + 1) * 128)
                        pss = ps_big.tile([128, S], F32, tag="ps", name="pss")
                        mmh(pss, k_t[h_][:, ksl], q_t[h_],
                            start=True, stop=True)
                        nc.scalar.activation(out=et, in_=pss, func=AF.Exp)
                        e_t.append(et)
                    pav = ps_big.tile([DH + 1, S], F32, tag="ps", name="pav")
                    for ktc in range(NTC):
                        mmh(pav, v_t[ktc][:, h_, :], e_t[ktc],
                            start=(ktc == 0), stop=(ktc == NTC - 1))
                    # 1/denom = exp(-ln(denom)) on ACT, same table set as Exp
                    rr = p_sm.tile([1, S], F32, tag="rr", name="rr")
                    nc.scalar.activation(out=rr, in_=pav[DH:DH + 1, :],
                                         func=AF.Ln)
                    rrb = p_sm.tile([1, S], BF16, tag="rrb", name="rrb")
                    nc.scalar.activation(out=rrb, in_=rr, func=AF.Exp,
                                         scale=-1.0)
                    prb = ps_big.tile([64, S], F32, tag="ps", name="prb")
                    mmh(prb, ones_bf[0:1, 0:64], rrb, start=True, stop=True)
                    rbs = p_sm.tile([64, S], BF16, tag="rbs", name="rbs")
                    nc.scalar.copy(out=rbs, in_=prb)
                    base = (h_ % 2) * 64
                    nc.vector.scalar_tensor_tensor(
                        out=ao_st[h_ // 2][base:base + 64, :],
                        in0=pav[0:DH, :], scalar=1.0, in1=rbs,
                        op0=OP.mult, op1=OP.mult)

                # o-projection + residual (contraction 128 over head pairs)
                wo = W["wo_w"]
                bto = BIA["wo_b"]
                for mc in range(NDC):
                    ps = ps_big.tile([128, S], F32, tag="ps", name="ps")
                    for i in range(4):
                        mmh(ps, wo[:, i, mc, :], ao_st[i],
                            start=(i == 0), stop=(i == 3))
                    resid_evict(bto)(ps, mc)

        # ================= conv module =================
        def conv_module():
            xs = ln_stats()
            with ExitStack() as ph:
                p_hp = ph.enter_context(tc.tile_pool(name="p_hp", bufs=NCC))
                # hp_odd and ca share one pool: hpo[pc] dies right before
                # ca[pc] is allocated, so NCC slots suffice for both.
                p_cah = ph.enter_context(tc.tile_pool(name="p_cah", bufs=NCC))
                p_acc = ph.enter_context(tc.tile_pool(name="p_acc", bufs=2))
                p_dg = (ph.enter_context(tc.tile_pool(name="p_dg", bufs=2))
                        if TAPS_PE else None)

                bt_a = BIA["pw1_b"]
                dwb = BIA["dw_b"]
                dww = W["dw_w"]

                hp_t, hpo_t = [], []
                for pc in range(NCC):
                    wt_a = p_pw1.tile([128, NDC, 128], BF16, tag="pw",
                                      name="wt_a")
                    nc.sync.dma_start(out=wt_a, in_=dram["pw1_w"][:, pc, :, :])
                    wt_g = p_pw1.tile([128, NDC, 128], BF16, tag="pw",
                                      name="wt_g")
                    nc.sync.dma_start(out=wt_g,
                                      in_=dram["pw1_w"][:, pc + NCC, :, :])
                    hp = p_hp.tile([128, S + 2 * PAD], BF16, tag="hp",
                                   name="hp")
                    nc.vector.memset(hp[:, 0:PAD], 0.0)
                    nc.vector.memset(hp[:, PAD + S:], 0.0)
                    psa = ps_big.tile([128, S], F32, tag="ps", name="psa")
                    psg = ps_big.tile([128, S], F32, tag="ps", name="psg")
                    for kc in range(NDC):
                        mmh(psa, wt_a[:, kc, :], xs[kc],
                            start=(kc == 0), stop=(kc == NDC - 1))
                    for kc in range(NDC):
                        mmh(psg, wt_g[:, kc, :], xs[kc],
                            start=(kc == 0), stop=(kc == NDC - 1))
                    sig = p_acc.tile([128, S], BF16, tag="sig", name="sig")
                    nc.scalar.activation(out=sig, in_=psg, func=AF.Sigmoid,
                                         bias=bt_a[:, pc + NCC:pc + NCC + 1],
                                         scale=1.0)
                    nc.vector.scalar_tensor_tensor(
                        out=hp[:, PAD:PAD + S], in0=psa,
                        scalar=bt_a[:, pc:pc + 1], in1=sig,
                        op0=OP.add, op1=OP.mult)
                    hp_t.append(hp)
                    if NEED_ODD:
                        ho = p_cah.tile([128, S + 2 * PAD], BF16, tag="hc",
                                        name="hpo")
                        nc.vector.tensor_copy(
                            out=ho[:, 0:S + 2 * PAD - 1],
                            in_=hp[:, 1:S + 2 * PAD])
                        hpo_t.append(ho)

                # depthwise conv taps: PE diag + DVE + GPSIMD
                ca_t = []
                for pc in range(NCC):
                    hp = hp_t[pc]
                    acc = None
                    if TAPS_DVE:
                        acc = p_acc.tile([128, S], BF16, tag="acc", name="acc")
                        first = True
                        for j in TAPS_DVE:
                            if j % 2 == 0:
                                src = hp[:, j:j + S]
                            else:
                                src = hpo_t[pc][:, j - 1:j - 1 + S]
                            if first:
                                nc.vector.tensor_scalar_mul(
                                    out=acc, in0=src,
                                    scalar1=dww[:, pc, j:j + 1])
                                first = False
                            else:
                                nc.vector.scalar_tensor_tensor(
                                    out=acc, in0=src,
                                    scalar=dww[:, pc, j:j + 1], in1=acc,
                                    op0=OP.mult, op1=OP.add)
                    accg = None
                    if TAPS_GPS:
                        accg = p_acc.tile([128, S], BF16, tag="accg",
                                          name="accg")
                        first = True
                        for j in TAPS_GPS:
                            if first:
                                nc.gpsimd.tensor_scalar_mul(
                                    out=accg, in0=hp[:, j:j + S],
                                    scalar1=dww[:, pc, j:j + 1])
                                first = False
                            else:
                                nc.gpsimd.scalar_tensor_tensor(
                                    out=accg, in0=hp[:, j:j + S],
                                    scalar=dww[:, pc, j:j + 1], in1=accg,
                                    op0=OP.mult, op1=OP.add)
                    if acc is not None and accg is not None:
                        nc.vector.tensor_tensor(out=acc, in0=acc, in1=accg,
                                                op=OP.add)
                    elif acc is None:
                        acc = accg
                    ca = p_cah.tile([128, S + 2 * PAD], BF16, tag="hc",
                                    name="ca")
                    ca = ca[:, 0:S]
                    if TAPS_PE:
                        dgt = p_dg.tile([128, len(TAPS_PE), 128], BF16,
                                        tag="dg", name="dg")
                        nc.sync.dma_start(out=dgt,
                                          in_=dram["dw_diag"][:, pc, :, :])
                        psc = ps_big.tile([128, S], F32, tag="ps", name="psc")
                        for jj, j in enumerate(TAPS_PE):
                            mmh(psc, dgt[:, jj, :], hp[:, j:j + S],
                                start=(jj == 0),
                                stop=(jj == len(TAPS_PE) - 1))
                        if acc is not None:
                            tmp = p_acc.tile([128, S], BF16, tag="tmp",
                                             name="tmp", bufs=2)
                            nc.vector.scalar_tensor_tensor(
                                out=tmp, in0=psc, scalar=0.0, in1=acc,
                                op0=OP.add, op1=OP.add)
                        else:
                            tmp = psc
                        nc.scalar.activation(out=ca, in_=tmp, func=AF.Silu,
                                             bias=dwb[:, pc:pc + 1], scale=1.0)
                    else:
                        nc.scalar.activation(out=ca, in_=acc, func=AF.Silu,
                                             bias=dwb[:, pc:pc + 1], scale=1.0)
                    ca_t.append(ca)

                dense(W["pw2_w"], NCC, NDC, ca_t, resid_evict(BIA["pw2_b"]))

        # ================= run the block =================
        _mods = os.environ.get("CONF_MODULES", "ffacf")
        if "f" in _mods:
            ff_module("ff1a", "ff1b")
            dbg_dump("ff1")
        print("built ff1", flush=True)
        if "a" in _mods:
            attn_module()
            dbg_dump("attn")
        print("built attn", flush=True)
        if "c" in _mods:
            conv_module()
            dbg_dump("conv")
        print("built conv", flush=True)
        if _mods.count("f") > 1:
            ff_module("ff2a", "ff2b")
            dbg_dump("ff2")
        print("built ff2", flush=True)

        # ---- final LN + transpose out ----
        xs_f = ln_stats()
        gt, bt = BIA["outln_g"], BIA["outln_b"]
        if not TRIV_FINAL:
            for mc in range(NDC):
                nc.vector.tensor_scalar(
                    out=xs_f[mc], in0=xs_f[mc],
                    scalar1=gt[:, mc:mc + 1], scalar2=bt[:, mc:mc + 1],
                    op0=OP.mult, op1=OP.add)
        with tc.tile_pool(name="p_out", bufs=4) as p_out:
            for tck in range(NTC):
                pt = ps_st.tile([128, D], BF16, tag="st", name="pt")
                tsl = slice(tck * 128, (tck + 1) * 128)
                for mc in range(NDC):
                    nc.tensor.transpose(out=pt[:, mc * 128:(mc + 1) * 128],
                                        in_=xs_f[mc][:, tsl],
                                        identity=ident_bf)
                ob = p_out.tile([128, D], F32, tag="ob", name="ob")
                nc.scalar.copy(out=ob, in_=pt)
                nc.sync.dma_start(out=y_d[tsl, :], in_=ob)

    _split_excess_waits(nc)
    return nc


def _split_excess_waits(nc, limit=1):
    """This walrus build caps sync-waits per instruction very low; hoist
    excess waits onto single-wait NOPs inserted before the instruction on
    the same engine (same-engine program order preserves the guarantee)."""
    from concourse import mybir
    cnt = 0
    for fn in nc.m.functions:
        for bb in fn.blocks:
            out = []
            for ins in bb.instructions:
                si = getattr(ins, "sync_info", None)
                if si is not None and si.on_wait and len(si.on_wait) > limit:
                    waits = list(si.on_wait)
                    keep = waits[:limit]
                    for w in waits[limit:]:
                        cnt += 1
                        out.append(mybir.InstNoOp(
                            name=f"waitnop_{cnt}",
                            engine=ins.engine,
                            sync_info=mybir.SyncInfo(on_wait=[w],
                                                     on_update=[]),
                        ))
                    si.on_wait = keep
                out.append(ins)
            bb.instructions = out
    return cnt


_CACHE = {}


def _get_program(has_vb, triv_final):
    key = (N_PE_TAPS, N_GPS_TAPS, DBG, has_vb, triv_final,
           os.environ.get("CONF_MODULES", "ffacf"))
    if key not in _CACHE:
        build_program._has_vb = has_vb
        build_program._triv_final = triv_final
        _CACHE[key] = build_program()
    return _CACHE[key]


LAST_EXEC_NS = None


def kernel(**inputs):
    global LAST_EXEC_NS
    from concourse.bass_utils import run_bass_kernel_spmd

    w = prep_inputs(inputs)
    has_vb = w.pop("_has_vb")
    triv_final = w.pop("_triv_final")
    nc = _get_program(has_vb, triv_final)

    x = np.asarray(inputs["x"], np.float32)
    in_maps = []
    for c in range(NCORES):
        m = dict(w)
        m["x"] = np.ascontiguousarray(x[c])
        in_maps.append(m)
    trace = os.environ.get("CONF_TRACE", "0") == "1"
    res = run_bass_kernel_spmd(nc, in_maps, core_ids=list(range(NCORES)),
                               trace=trace)
    LAST_EXEC_NS = res.exec_time_ns
    out = np.stack([res.results[c]["y"] for c in range(NCORES)], 0)
    return out.astype(np.float32)
